# revision 15
# baseline (speedup 1.0000x reference)
"""Blended-MoE 3-layer MLP (nn_Expert) on 8 Trainium2 NeuronCores.

Math: per layer, y[b,o] = act( sum_e blend[b,e] * (W[e] @ x[b] + B[e])[o] ).
Rewritten as a dense matmul with a per-expert prescale of the activations:
  y[o,b] = act( sum_e sum_k Wf[e*I+k, o] * (blend[e,b] * hT[k,b]) + bias )
where Wf[(e,i), o] = W[e,o,i] and everything is kept transposed on-chip
([feature, batch] layout) so each layer's output feeds the next directly.

Sharding: data-parallel over the batch — 4096 tokens -> 512 per core; the
per-expert weight stacks are replicated. Matmuls run as float32r (full PE
rate, ~3e-4 rel err). PSUM accumulates over experts and the contraction; the
bias enters as one extra K=8 matmul (blend.T as rhs) per output tile.
Activations are stored 4 k-blocks wide ([128, 2048]) so one DVE prescale op
(with a free-dim-broadcast blend operand) feeds 4 k-tiles of matmuls.
Weights are pre-split by o-half on the host so every tile DMA is one
contiguous 512KB block.
"""

import os

import numpy as np
import ml_dtypes

import concourse.bass as bass
import concourse.tile as tile
import concourse.mybir as mybir
from concourse import bacc
from concourse.bass_utils import run_bass_kernel_spmd
from contextlib import ExitStack

dt = mybir.dt
ALU = mybir.AluOpType
ACTF = mybir.ActivationFunctionType

N_CORES = 8
B_FULL = 4096
BC = B_FULL // N_CORES  # 512 tokens per core
E = 8
DIMS = [1024, 2048, 2048, 512]
LAYERS = [  # (I, O, has_elu)
    (1024, 2048, True),
    (2048, 2048, True),
    (2048, 512, False),
]
OH = 1024   # o-columns per half-pass (8 psum banks)
GW = 8      # k-blocks packed per wide activation tile
WIDE = GW * BC

USE_BF16 = os.environ.get("MOE_MM_DT", "f32r") == "bf16"
MM_DT = dt.bfloat16 if USE_BF16 else dt.float32r
MM_NP = ml_dtypes.bfloat16 if USE_BF16 else np.float32

_cache = {}


def _build():
    nc = bacc.Bacc("TRN2", target_bir_lowering=False, debug=False,
                   num_devices=N_CORES)
    xTw = nc.declare_dram_parameter("xTw", [DIMS[0] // (128 * GW), 128, WIDE],
                                    dt.float32, isOutput=False)
    blT = nc.declare_dram_parameter("blT", [E, BC], MM_DT, isOutput=False)
    brep = nc.declare_dram_parameter("brep", [E, 128, BC], dt.float32, isOutput=False)
    # weights pre-split by o-half on the host so every [128, width] tile DMA
    # reads one fully contiguous block
    wf = [nc.declare_dram_parameter(f"w{l}f",
                                    [max(O // OH, 1), E * I, min(OH, O)],
                                    MM_DT, isOutput=False)
          for l, (I, O, _) in enumerate(LAYERS)]
    bf = [nc.declare_dram_parameter(f"b{l}f", [E, O], MM_DT, isOutput=False)
          for l, (I, O, _) in enumerate(LAYERS)]
    yT = nc.declare_dram_parameter("yT", [DIMS[3], BC], dt.float32, isOutput=True)

    tc = tile.TileContext(nc)
    with tc:
        with ExitStack() as ctx:
            const = ctx.enter_context(tc.tile_pool(name="const", bufs=1))
            act = ctx.enter_context(tc.tile_pool(name="act", bufs=1))
            xpool = ctx.enter_context(tc.tile_pool(name="xpool", bufs=2))
            wpool = ctx.enter_context(tc.tile_pool(name="wpool", bufs=8))
            tpool = ctx.enter_context(tc.tile_pool(name="tpool", bufs=2))
            ypool = ctx.enter_context(tc.tile_pool(name="ypool", bufs=2))
            pp = ctx.enter_context(tc.tile_pool(name="pp", bufs=8, space="PSUM"))

            blT_sb = const.tile([E, BC], MM_DT, tag="blT")
            nc.sync.dma_start(blT_sb[:], blT[:])
            bf_sb = [None, None, None]
            brep_sb = [None] * E

            # layer 0 input, packed 4 k-blocks wide; tile g loaded lazily at
            # first use so startup only waits for tile 0
            hT = [None] * (DIMS[0] // (128 * GW))

            def load_x(g):
                t = act.tile([128, WIDE], dt.float32, name=f"h0_{g}", tag=f"h0_{g}")
                nc.sync.dma_start(t[:], xTw[g, :, :])
                hT[g] = t

            pending_drain = []  # deferred drain emission from the previous half

            for l, (I, O, has_elu) in enumerate(LAYERS):
                NG = I // (128 * GW)  # wide groups per layer input
                t = const.tile([E, O], MM_DT, tag=f"bf{l}")
                nc.sync.dma_start(t[:], bf[l][:])
                bf_sb[l] = t
                h_next = []
                for g in range(O * BC // (128 * WIDE)):
                    h_next.append(act.tile([128, WIDE], dt.float32,
                                           name=f"h{l + 1}_{g}",
                                           tag=f"h{l + 1}_{g}"))
                for half_start in range(0, O, OH):
                    width = min(OH, O - half_start)
                    n_ot = width // 128
                    # open accumulation groups with the bias matmul (K=8)
                    ps = []
                    for j in range(n_ot):
                        p = pp.tile([128, BC], dt.float32, tag="ps")
                        nc.tensor.matmul(
                            p[:],
                            bf_sb[l][:, half_start + j * 128:half_start + (j + 1) * 128],
                            blT_sb[:],
                            start=True, stop=False)
                        ps.append(p)
                    # stream experts x wide k-groups
                    for e in range(E):
                        if brep_sb[e] is None:
                            t = const.tile([128, BC], dt.float32, tag=f"brep{e}")
                            nc.sync.dma_start(t[:], brep[e, :, :])
                            brep_sb[e] = t
                        for g in range(NG):
                            if l == 0 and hT[g] is None:
                                load_x(g)
                            xp = xpool.tile([128, WIDE], MM_DT, tag="xp")
                            nc.vector.tensor_tensor(
                                xp[:].rearrange("p (c b) -> p c b", c=GW),
                                hT[g][:].rearrange("p (c b) -> p c b", c=GW),
                                brep_sb[e][:].unsqueeze(1).broadcast_to((128, GW, BC)),
                                ALU.mult)
                            last_g = (e == E - 1 and g == NG - 1)
                            wts = []
                            for c in range(GW):
                                kt = g * GW + c
                                wt = wpool.tile([128, width], MM_DT, tag="wt")
                                row = e * I + kt * 128
                                nc.sync.dma_start(
                                    wt[:], wf[l][half_start // OH,
                                                 row:row + 128, :])
                                wts.append(wt)
                                if not last_g:
                                    for j in range(n_ot):
                                        nc.tensor.matmul(
                                            ps[j][:], wt[:, j * 128:(j + 1) * 128],
                                            xp[:, c * BC:(c + 1) * BC],
                                            start=False, stop=False)
                            if last_g:
                                # final group: j-outer so each bank's group
                                # stops early and its drain overlaps the rest
                                for j in range(n_ot):
                                    for c in range(GW):
                                        nc.tensor.matmul(
                                            ps[j][:], wts[c][:, j * 128:(j + 1) * 128],
                                            xp[:, c * BC:(c + 1) * BC],
                                            start=False, stop=(c == GW - 1))
                            if e == 0 and g == 0 and pending_drain:
                                # emit the previous half's drains only after
                                # this half's first wave, so the scheduler
                                # prioritizes restarting the PE pipeline
                                for fn in pending_drain:
                                    fn()
                                pending_drain = []
                    # defer drain emission
                    def make_drain(l, has_elu, half_start, n_ot, ps, h_next):
                        def drain():
                            for j in range(n_ot):
                                ot = (half_start + j * 128) // 128
                                if has_elu:
                                    # elu(v) = relu(v) + exp(min(v,0)) - 1
                                    m = tpool.tile([128, BC], dt.float32, tag="m")
                                    nc.vector.tensor_scalar_min(m[:], ps[j][:], 0.0)
                                    r = tpool.tile([128, BC], dt.float32, tag="r")
                                    nc.scalar.activation(r[:], ps[j][:], ACTF.Relu)
                                    x2 = tpool.tile([128, BC], dt.float32, tag="x2")
                                    nc.scalar.activation(x2[:], m[:], ACTF.Exp)
                                    dst = h_next[ot // GW][
                                        :, (ot % GW) * BC:(ot % GW + 1) * BC]
                                    nc.vector.scalar_tensor_tensor(
                                        dst, x2[:], -1.0, r[:], ALU.add, ALU.add)
                                else:
                                    y = ypool.tile([128, BC], dt.float32, tag="y")
                                    nc.vector.tensor_copy(y[:], ps[j][:])
                                    nc.sync.dma_start(
                                        yT[half_start + j * 128:
                                           half_start + (j + 1) * 128, :],
                                        y[:])
                        return drain
                    pending_drain.append(
                        make_drain(l, has_elu, half_start, n_ot, ps, h_next))
                hT = h_next
            for fn in pending_drain:
                fn()
    nc.compile()
    return nc


def _prep_inputs(weight_blend, x, W0, B0, W1, B1, W2, B2):
    Ws = [W0, W1, W2]
    Bs = [B0, B1, B2]
    shared = {}
    for l in range(3):
        I, O, _ = LAYERS[l]
        wfl = Ws[l].transpose(0, 2, 1).reshape(E * I, O)
        nh = max(O // OH, 1)
        shared[f"w{l}f"] = np.ascontiguousarray(
            np.stack([wfl[:, h * OH:h * OH + min(OH, O)] for h in range(nh)]),
            dtype=MM_NP)
        shared[f"b{l}f"] = np.ascontiguousarray(Bs[l][:, :, 0], dtype=MM_NP)
    in_maps = []
    for c in range(N_CORES):
        s = slice(c * BC, (c + 1) * BC)
        blT = np.ascontiguousarray(weight_blend[s].T, dtype=np.float32)
        m = dict(shared)
        # pack x.T into [NG, 128, GW*BC] wide tiles: block kt = g*GW + c
        xt = np.ascontiguousarray(x[s].T, dtype=np.float32)  # [1024, 512]
        m["xTw"] = np.ascontiguousarray(
            xt.reshape(-1, GW, 128, BC).transpose(0, 2, 1, 3).reshape(-1, 128, WIDE))
        m["blT"] = blT.astype(MM_NP)
        m["brep"] = np.ascontiguousarray(
            np.broadcast_to(blT[:, None, :], (E, 128, BC)), dtype=np.float32)
        in_maps.append(m)
    return in_maps


def run(inputs, trace=False, tmpdir=None, trace_cores=None):
    """Run on hardware; returns (y, BassKernelResults)."""
    if "nc" not in _cache:
        _cache["nc"] = _build()
    nc = _cache["nc"]
    in_maps = _prep_inputs(**inputs)
    kw = {}
    if tmpdir:
        kw["tmpdir"] = tmpdir
    if trace_cores:
        kw["trace_cores"] = trace_cores
    res = run_bass_kernel_spmd(
        nc, in_maps, core_ids=list(range(N_CORES)), trace=trace, **kw)
    y = np.concatenate([r["yT"].T for r in res.results], axis=0)
    return np.ascontiguousarray(y, dtype=np.float32), res


def kernel(**inputs):
    y, _ = run(inputs, trace=False)
    return y


# revision 16
# speedup vs baseline: 1.0145x; 1.0145x over previous
"""Blended-MoE 3-layer MLP (nn_Expert) on 8 Trainium2 NeuronCores.

Math: per layer, y[b,o] = act( sum_e blend[b,e] * (W[e] @ x[b] + B[e])[o] ).
Rewritten as a dense matmul with a per-expert prescale of the activations:
  y[o,b] = act( sum_e sum_k Wf[e*I+k, o] * (blend[e,b] * hT[k,b]) + bias )
where Wf[(e,i), o] = W[e,o,i] and everything is kept transposed on-chip
([feature, batch] layout) so each layer's output feeds the next directly.

Sharding: data-parallel over the batch — 4096 tokens -> 512 per core; the
per-expert weight stacks are replicated. Matmuls run as float32r (full PE
rate, ~3e-4 rel err). PSUM accumulates over experts and the contraction; the
bias enters as one extra K=8 matmul (blend.T as rhs) per output tile.
Activations are stored 4 k-blocks wide ([128, 2048]) so one DVE prescale op
(with a free-dim-broadcast blend operand) feeds 4 k-tiles of matmuls.
Weights are pre-split by o-half on the host so every tile DMA is one
contiguous 512KB block.
"""

import os

import numpy as np
import ml_dtypes

import concourse.bass as bass
import concourse.tile as tile
import concourse.mybir as mybir
from concourse import bacc
from concourse.bass_utils import run_bass_kernel_spmd
from contextlib import ExitStack

dt = mybir.dt
ALU = mybir.AluOpType
ACTF = mybir.ActivationFunctionType

N_CORES = 8
B_FULL = 4096
BC = B_FULL // N_CORES  # 512 tokens per core
E = 8
DIMS = [1024, 2048, 2048, 512]
LAYERS = [  # (I, O, has_elu)
    (1024, 2048, True),
    (2048, 2048, True),
    (2048, 512, False),
]
OH = 1024   # o-columns per half-pass (8 psum banks)
GW = 4      # k-blocks packed per wide activation tile
WIDE = GW * BC

USE_BF16 = os.environ.get("MOE_MM_DT", "f32r") == "bf16"
MM_DT = dt.bfloat16 if USE_BF16 else dt.float32r
MM_NP = ml_dtypes.bfloat16 if USE_BF16 else np.float32

_cache = {}


def _build():
    nc = bacc.Bacc("TRN2", target_bir_lowering=False, debug=False,
                   num_devices=N_CORES)
    xTw = nc.declare_dram_parameter("xTw", [DIMS[0] // (128 * GW), 128, WIDE],
                                    dt.float32, isOutput=False)
    blT = nc.declare_dram_parameter("blT", [E, BC], MM_DT, isOutput=False)
    brep = nc.declare_dram_parameter("brep", [E, 128, BC], dt.float32, isOutput=False)
    # weights pre-split by o-half on the host so every [128, width] tile DMA
    # reads one fully contiguous block
    wf = [nc.declare_dram_parameter(f"w{l}f",
                                    [max(O // OH, 1), E * I, min(OH, O)],
                                    MM_DT, isOutput=False)
          for l, (I, O, _) in enumerate(LAYERS)]
    bf = [nc.declare_dram_parameter(f"b{l}f", [E, O], MM_DT, isOutput=False)
          for l, (I, O, _) in enumerate(LAYERS)]
    yT = nc.declare_dram_parameter("yT", [DIMS[3], BC], dt.float32, isOutput=True)

    tc = tile.TileContext(nc)
    with tc:
        with ExitStack() as ctx:
            const = ctx.enter_context(tc.tile_pool(name="const", bufs=1))
            act = ctx.enter_context(tc.tile_pool(name="act", bufs=1))
            xpool = ctx.enter_context(tc.tile_pool(name="xpool", bufs=3))
            wpool = ctx.enter_context(tc.tile_pool(name="wpool", bufs=10))
            tpool = ctx.enter_context(tc.tile_pool(name="tpool", bufs=2))
            ypool = ctx.enter_context(tc.tile_pool(name="ypool", bufs=2))
            pp = ctx.enter_context(tc.tile_pool(name="pp", bufs=8, space="PSUM"))

            blT_sb = const.tile([E, BC], MM_DT, tag="blT")
            nc.sync.dma_start(blT_sb[:], blT[:])
            bf_sb = [None, None, None]
            brep_sb = [None] * E

            # layer 0 input, packed 4 k-blocks wide; tile g loaded lazily at
            # first use so startup only waits for tile 0
            hT = [None] * (DIMS[0] // (128 * GW))

            def load_x(g):
                t = act.tile([128, WIDE], dt.float32, name=f"h0_{g}", tag=f"h0_{g}")
                nc.sync.dma_start(t[:], xTw[g, :, :])
                hT[g] = t

            pending_drain = []  # deferred drain emission from the previous half

            for l, (I, O, has_elu) in enumerate(LAYERS):
                NG = I // (128 * GW)  # wide groups per layer input
                t = const.tile([E, O], MM_DT, tag=f"bf{l}")
                nc.sync.dma_start(t[:], bf[l][:])
                bf_sb[l] = t
                h_next = []
                for g in range(O * BC // (128 * WIDE)):
                    h_next.append(act.tile([128, WIDE], dt.float32,
                                           name=f"h{l + 1}_{g}",
                                           tag=f"h{l + 1}_{g}"))
                for half_start in range(0, O, OH):
                    width = min(OH, O - half_start)
                    n_ot = width // 128
                    # open accumulation groups with the bias matmul (K=8)
                    ps = []
                    for j in range(n_ot):
                        p = pp.tile([128, BC], dt.float32, tag="ps")
                        nc.tensor.matmul(
                            p[:],
                            bf_sb[l][:, half_start + j * 128:half_start + (j + 1) * 128],
                            blT_sb[:],
                            start=True, stop=False)
                        ps.append(p)
                    # stream experts x wide k-groups
                    for e in range(E):
                        if brep_sb[e] is None:
                            t = const.tile([128, BC], dt.float32, tag=f"brep{e}")
                            nc.sync.dma_start(t[:], brep[e, :, :])
                            brep_sb[e] = t
                        for g in range(NG):
                            if l == 0 and hT[g] is None:
                                load_x(g)
                            xp = xpool.tile([128, WIDE], MM_DT, tag="xp")
                            nc.vector.tensor_tensor(
                                xp[:].rearrange("p (c b) -> p c b", c=GW),
                                hT[g][:].rearrange("p (c b) -> p c b", c=GW),
                                brep_sb[e][:].unsqueeze(1).broadcast_to((128, GW, BC)),
                                ALU.mult)
                            last_g = (e == E - 1 and g == NG - 1)
                            wts = []
                            for c in range(GW):
                                kt = g * GW + c
                                wt = wpool.tile([128, width], MM_DT, tag="wt")
                                row = e * I + kt * 128
                                nc.sync.dma_start(
                                    wt[:], wf[l][half_start // OH,
                                                 row:row + 128, :])
                                wts.append(wt)
                                if not last_g:
                                    for j in range(n_ot):
                                        nc.tensor.matmul(
                                            ps[j][:], wt[:, j * 128:(j + 1) * 128],
                                            xp[:, c * BC:(c + 1) * BC],
                                            start=False, stop=False)
                            if last_g:
                                # final group: j-outer so each bank's group
                                # stops early and its drain overlaps the rest
                                for j in range(n_ot):
                                    for c in range(GW):
                                        nc.tensor.matmul(
                                            ps[j][:], wts[c][:, j * 128:(j + 1) * 128],
                                            xp[:, c * BC:(c + 1) * BC],
                                            start=False, stop=(c == GW - 1))
                            if e == 0 and g == 0 and pending_drain:
                                # emit the previous half's drains only after
                                # this half's first wave, so the scheduler
                                # prioritizes restarting the PE pipeline
                                for fn in pending_drain:
                                    fn()
                                pending_drain = []
                    # defer drain emission
                    def make_drain(l, has_elu, half_start, n_ot, ps, h_next):
                        def drain():
                            for j in range(n_ot):
                                ot = (half_start + j * 128) // 128
                                if has_elu:
                                    # elu(v) = relu(v) + exp(min(v,0)) - 1
                                    m = tpool.tile([128, BC], dt.float32, tag="m")
                                    nc.vector.tensor_scalar_min(m[:], ps[j][:], 0.0)
                                    r = tpool.tile([128, BC], dt.float32, tag="r")
                                    nc.scalar.activation(r[:], ps[j][:], ACTF.Relu)
                                    x2 = tpool.tile([128, BC], dt.float32, tag="x2")
                                    nc.scalar.activation(x2[:], m[:], ACTF.Exp)
                                    dst = h_next[ot // GW][
                                        :, (ot % GW) * BC:(ot % GW + 1) * BC]
                                    nc.vector.scalar_tensor_tensor(
                                        dst, x2[:], -1.0, r[:], ALU.add, ALU.add)
                                else:
                                    y = ypool.tile([128, BC], dt.float32, tag="y")
                                    nc.vector.tensor_copy(y[:], ps[j][:])
                                    nc.sync.dma_start(
                                        yT[half_start + j * 128:
                                           half_start + (j + 1) * 128, :],
                                        y[:])
                        return drain
                    pending_drain.append(
                        make_drain(l, has_elu, half_start, n_ot, ps, h_next))
                hT = h_next
            for fn in pending_drain:
                fn()
    nc.compile()
    return nc


def _prep_inputs(weight_blend, x, W0, B0, W1, B1, W2, B2):
    Ws = [W0, W1, W2]
    Bs = [B0, B1, B2]
    shared = {}
    for l in range(3):
        I, O, _ = LAYERS[l]
        wfl = Ws[l].transpose(0, 2, 1).reshape(E * I, O)
        nh = max(O // OH, 1)
        shared[f"w{l}f"] = np.ascontiguousarray(
            np.stack([wfl[:, h * OH:h * OH + min(OH, O)] for h in range(nh)]),
            dtype=MM_NP)
        shared[f"b{l}f"] = np.ascontiguousarray(Bs[l][:, :, 0], dtype=MM_NP)
    in_maps = []
    for c in range(N_CORES):
        s = slice(c * BC, (c + 1) * BC)
        blT = np.ascontiguousarray(weight_blend[s].T, dtype=np.float32)
        m = dict(shared)
        # pack x.T into [NG, 128, GW*BC] wide tiles: block kt = g*GW + c
        xt = np.ascontiguousarray(x[s].T, dtype=np.float32)  # [1024, 512]
        m["xTw"] = np.ascontiguousarray(
            xt.reshape(-1, GW, 128, BC).transpose(0, 2, 1, 3).reshape(-1, 128, WIDE))
        m["blT"] = blT.astype(MM_NP)
        m["brep"] = np.ascontiguousarray(
            np.broadcast_to(blT[:, None, :], (E, 128, BC)), dtype=np.float32)
        in_maps.append(m)
    return in_maps


def run(inputs, trace=False, tmpdir=None, trace_cores=None):
    """Run on hardware; returns (y, BassKernelResults)."""
    if "nc" not in _cache:
        _cache["nc"] = _build()
    nc = _cache["nc"]
    in_maps = _prep_inputs(**inputs)
    kw = {}
    if tmpdir:
        kw["tmpdir"] = tmpdir
    if trace_cores:
        kw["trace_cores"] = trace_cores
    res = run_bass_kernel_spmd(
        nc, in_maps, core_ids=list(range(N_CORES)), trace=trace, **kw)
    y = np.concatenate([r["yT"].T for r in res.results], axis=0)
    return np.ascontiguousarray(y, dtype=np.float32), res


def kernel(**inputs):
    y, _ = run(inputs, trace=False)
    return y


# revision 23
# speedup vs baseline: 1.0302x; 1.0155x over previous
"""Blended-MoE 3-layer MLP (nn_Expert) on 8 Trainium2 NeuronCores.

Math: per layer, y[b,o] = act( sum_e blend[b,e] * (W[e] @ x[b] + B[e])[o] ).
Rewritten as a dense matmul with a per-expert prescale of the activations:
  y[o,b] = act( sum_e sum_k Wf[e*I+k, o] * (blend[e,b] * hT[k,b]) + bias )
where Wf[(e,i), o] = W[e,o,i] and everything is kept transposed on-chip
([feature, batch] layout) so each layer's output feeds the next directly.

Sharding: data-parallel over the batch — 4096 tokens -> 512 per core; the
per-expert weight stacks are replicated. Matmuls run as float32r (full PE
rate, ~3e-4 rel err). PSUM accumulates over experts and the contraction; the
bias enters as one extra K=8 matmul (blend.T as rhs) per output tile.
Activations are stored 4 k-blocks wide ([128, 2048]) so one DVE prescale op
(with a free-dim-broadcast blend operand) feeds 4 k-tiles of matmuls.
Weights are pre-split by o-half on the host so every tile DMA is one
contiguous 512KB block.
"""

import os

import numpy as np
import ml_dtypes

import concourse.bass as bass
import concourse.tile as tile
import concourse.mybir as mybir
from concourse import bacc
from concourse.bass_utils import run_bass_kernel_spmd
from contextlib import ExitStack

dt = mybir.dt
ALU = mybir.AluOpType
ACTF = mybir.ActivationFunctionType

N_CORES = 8
B_FULL = 4096
BC = B_FULL // N_CORES  # 512 tokens per core
E = 8
DIMS = [1024, 2048, 2048, 512]
LAYERS = [  # (I, O, has_elu)
    (1024, 2048, True),
    (2048, 2048, True),
    (2048, 512, False),
]
OH = 1024   # o-columns per half-pass (8 psum banks)
GW = 4      # k-blocks packed per wide activation tile
WIDE = GW * BC

USE_BF16 = os.environ.get("MOE_MM_DT", "f32r") == "bf16"
MM_DT = dt.bfloat16 if USE_BF16 else dt.float32r
MM_NP = ml_dtypes.bfloat16 if USE_BF16 else np.float32

_cache = {}


def _build(with_bias=True):
    nc = bacc.Bacc("TRN2", target_bir_lowering=False, debug=False,
                   num_devices=N_CORES)
    xTw = nc.declare_dram_parameter("xTw", [DIMS[0] // (128 * GW), 128, WIDE],
                                    dt.float32, isOutput=False)
    blT = nc.declare_dram_parameter("blT", [E, BC], MM_DT, isOutput=False)
    brep = nc.declare_dram_parameter("brep", [E, 128, BC], dt.float32, isOutput=False)
    # weights pre-split by o-half on the host so every [128, width] tile DMA
    # reads one fully contiguous block
    wf = [nc.declare_dram_parameter(f"w{l}f",
                                    [max(O // OH, 1), E * I, min(OH, O)],
                                    MM_DT, isOutput=False)
          for l, (I, O, _) in enumerate(LAYERS)]
    bf = [nc.declare_dram_parameter(f"b{l}f", [E, O], MM_DT, isOutput=False)
          for l, (I, O, _) in enumerate(LAYERS)]
    yT = nc.declare_dram_parameter("yT", [DIMS[3], BC], dt.float32, isOutput=True)

    tc = tile.TileContext(nc)
    with tc:
        with ExitStack() as ctx:
            const = ctx.enter_context(tc.tile_pool(name="const", bufs=1))
            act = ctx.enter_context(tc.tile_pool(name="act", bufs=1))
            xpool = ctx.enter_context(tc.tile_pool(name="xpool", bufs=3))
            wpool = ctx.enter_context(tc.tile_pool(name="wpool", bufs=10))
            tpool = ctx.enter_context(tc.tile_pool(name="tpool", bufs=2))
            ypool = ctx.enter_context(tc.tile_pool(name="ypool", bufs=2))
            pp = ctx.enter_context(tc.tile_pool(name="pp", bufs=8, space="PSUM"))

            blT_sb = const.tile([E, BC], MM_DT, tag="blT")
            nc.sync.dma_start(blT_sb[:], blT[:])
            bf_sb = [None, None, None]
            brep_sb = [None] * E

            # warm the PE clock gate (HAM) with throwaway matmuls while the
            # first input/weight DMAs are in flight, so the real stream
            # starts at 2.4 GHz; the tile recycles into the first half's
            # bank set afterwards
            warm = pp.tile([128, BC], dt.float32, tag="ps")
            for _ in range(24):
                nc.tensor.matmul(warm[:], blT_sb[:, :128], blT_sb[:],
                                 start=True, stop=True)

            # layer 0 input, packed 4 k-blocks wide; tile g loaded lazily at
            # first use so startup only waits for tile 0 (which is itself
            # split into chunks so the first prescale can begin early)
            hT = [None] * (DIMS[0] // (128 * GW))

            def load_x(g):
                t = act.tile([128, WIDE], dt.float32, name=f"h0_{g}", tag=f"h0_{g}")
                if g == 0:
                    for c4 in range(GW):
                        nc.sync.dma_start(t[:, c4 * BC:(c4 + 1) * BC],
                                          xTw[g, :, c4 * BC:(c4 + 1) * BC])
                else:
                    nc.sync.dma_start(t[:], xTw[g, :, :])
                hT[g] = t

            pending_drain = []  # deferred drain emission from the previous half

            for l, (I, O, has_elu) in enumerate(LAYERS):
                NG = I // (128 * GW)  # wide groups per layer input
                if with_bias:
                    t = const.tile([E, O], MM_DT, tag=f"bf{l}")
                    nc.sync.dma_start(t[:], bf[l][:])
                    bf_sb[l] = t
                h_next = []
                for g in range(O * BC // (128 * WIDE)):
                    h_next.append(act.tile([128, WIDE], dt.float32,
                                           name=f"h{l + 1}_{g}",
                                           tag=f"h{l + 1}_{g}"))
                for half_start in range(0, O, OH):
                    width = min(OH, O - half_start)
                    n_ot = width // 128
                    # open accumulation groups with the bias matmul (K=8);
                    # without bias the first weight matmul opens the group
                    ps = []
                    for j in range(n_ot):
                        p = pp.tile([128, BC], dt.float32, tag="ps")
                        if with_bias:
                            nc.tensor.matmul(
                                p[:],
                                bf_sb[l][:, half_start + j * 128:
                                         half_start + (j + 1) * 128],
                                blT_sb[:],
                                start=True, stop=False)
                        ps.append(p)
                    # stream experts x wide k-groups
                    for e in range(E):
                        if brep_sb[e] is None:
                            t = const.tile([128, BC], dt.float32, tag=f"brep{e}")
                            nc.sync.dma_start(t[:], brep[e, :, :])
                            brep_sb[e] = t
                        for g in range(NG):
                            if l == 0 and hT[g] is None:
                                load_x(g)
                            xp = xpool.tile([128, WIDE], MM_DT, tag="xp")
                            if l == 0 and e == 0 and g == 0:
                                # sliced prescale: each chunk only waits for
                                # its own quarter of the xT DMA
                                for c4 in range(GW):
                                    nc.vector.tensor_tensor(
                                        xp[:, c4 * BC:(c4 + 1) * BC],
                                        hT[g][:, c4 * BC:(c4 + 1) * BC],
                                        brep_sb[e][:], ALU.mult)
                            else:
                                nc.vector.tensor_tensor(
                                    xp[:].rearrange("p (c b) -> p c b", c=GW),
                                    hT[g][:].rearrange("p (c b) -> p c b", c=GW),
                                    brep_sb[e][:].unsqueeze(1).broadcast_to(
                                        (128, GW, BC)),
                                    ALU.mult)
                            last_g = (e == E - 1 and g == NG - 1)
                            wts = []
                            for c in range(GW):
                                kt = g * GW + c
                                wt = wpool.tile([128, width], MM_DT, tag="wt")
                                row = e * I + kt * 128
                                nc.sync.dma_start(
                                    wt[:], wf[l][half_start // OH,
                                                 row:row + 128, :])
                                wts.append(wt)
                                opener = (not with_bias and e == 0 and g == 0
                                          and c == 0)
                                if not last_g:
                                    for j in range(n_ot):
                                        nc.tensor.matmul(
                                            ps[j][:], wt[:, j * 128:(j + 1) * 128],
                                            xp[:, c * BC:(c + 1) * BC],
                                            start=opener, stop=False)
                            if last_g:
                                # final group: j-outer so each bank's group
                                # stops early and its drain overlaps the rest
                                for j in range(n_ot):
                                    for c in range(GW):
                                        nc.tensor.matmul(
                                            ps[j][:], wts[c][:, j * 128:(j + 1) * 128],
                                            xp[:, c * BC:(c + 1) * BC],
                                            start=False, stop=(c == GW - 1))
                            if e == 0 and g == 0 and pending_drain:
                                # emit the previous half's drains only after
                                # this half's first wave, so the scheduler
                                # prioritizes restarting the PE pipeline
                                for fn in pending_drain:
                                    fn()
                                pending_drain = []
                    # defer drain emission
                    def make_drain(l, has_elu, half_start, n_ot, ps, h_next):
                        def drain():
                            for j in range(n_ot):
                                ot = (half_start + j * 128) // 128
                                if has_elu:
                                    # elu(v) = relu(v) + exp(min(v,0)) - 1
                                    m = tpool.tile([128, BC], dt.float32, tag="m")
                                    nc.vector.tensor_scalar_min(m[:], ps[j][:], 0.0)
                                    r = tpool.tile([128, BC], dt.float32, tag="r")
                                    nc.scalar.activation(r[:], ps[j][:], ACTF.Relu)
                                    x2 = tpool.tile([128, BC], dt.float32, tag="x2")
                                    nc.scalar.activation(x2[:], m[:], ACTF.Exp)
                                    dst = h_next[ot // GW][
                                        :, (ot % GW) * BC:(ot % GW + 1) * BC]
                                    nc.vector.scalar_tensor_tensor(
                                        dst, x2[:], -1.0, r[:], ALU.add, ALU.add)
                                else:
                                    y = ypool.tile([128, BC], dt.float32, tag="y")
                                    nc.vector.tensor_copy(y[:], ps[j][:])
                                    nc.sync.dma_start(
                                        yT[half_start + j * 128:
                                           half_start + (j + 1) * 128, :],
                                        y[:])
                        return drain
                    pending_drain.append(
                        make_drain(l, has_elu, half_start, n_ot, ps, h_next))
                hT = h_next
            for fn in pending_drain:
                fn()
    nc.compile()
    return nc


def _prep_inputs(weight_blend, x, W0, B0, W1, B1, W2, B2):
    Ws = [W0, W1, W2]
    Bs = [B0, B1, B2]
    shared = {}
    for l in range(3):
        I, O, _ = LAYERS[l]
        wfl = Ws[l].transpose(0, 2, 1).reshape(E * I, O)
        nh = max(O // OH, 1)
        shared[f"w{l}f"] = np.ascontiguousarray(
            np.stack([wfl[:, h * OH:h * OH + min(OH, O)] for h in range(nh)]),
            dtype=MM_NP)
        shared[f"b{l}f"] = np.ascontiguousarray(Bs[l][:, :, 0], dtype=MM_NP)
    in_maps = []
    for c in range(N_CORES):
        s = slice(c * BC, (c + 1) * BC)
        blT = np.ascontiguousarray(weight_blend[s].T, dtype=np.float32)
        m = dict(shared)
        # pack x.T into [NG, 128, GW*BC] wide tiles: block kt = g*GW + c
        xt = np.ascontiguousarray(x[s].T, dtype=np.float32)  # [1024, 512]
        m["xTw"] = np.ascontiguousarray(
            xt.reshape(-1, GW, 128, BC).transpose(0, 2, 1, 3).reshape(-1, 128, WIDE))
        m["blT"] = blT.astype(MM_NP)
        m["brep"] = np.ascontiguousarray(
            np.broadcast_to(blT[:, None, :], (E, 128, BC)), dtype=np.float32)
        in_maps.append(m)
    return in_maps


def run(inputs, trace=False, tmpdir=None, trace_cores=None):
    """Run on hardware; returns (y, BassKernelResults)."""
    with_bias = any(
        np.any(np.asarray(inputs[k])) for k in ("B0", "B1", "B2"))
    key = ("nc", with_bias)
    if key not in _cache:
        _cache[key] = _build(with_bias)
    nc = _cache[key]
    in_maps = _prep_inputs(**inputs)
    kw = {}
    if tmpdir:
        kw["tmpdir"] = tmpdir
    if trace_cores:
        kw["trace_cores"] = trace_cores
    res = run_bass_kernel_spmd(
        nc, in_maps, core_ids=list(range(N_CORES)), trace=trace, **kw)
    y = np.concatenate([r["yT"].T for r in res.results], axis=0)
    return np.ascontiguousarray(y, dtype=np.float32), res


def kernel(**inputs):
    y, _ = run(inputs, trace=False)
    return y


# revision 24
# speedup vs baseline: 1.0322x; 1.0020x over previous
"""Blended-MoE 3-layer MLP (nn_Expert) on 8 Trainium2 NeuronCores.

Math: per layer, y[b,o] = act( sum_e blend[b,e] * (W[e] @ x[b] + B[e])[o] ).
Rewritten as a dense matmul with a per-expert prescale of the activations:
  y[o,b] = act( sum_e sum_k Wf[e*I+k, o] * (blend[e,b] * hT[k,b]) + bias )
where Wf[(e,i), o] = W[e,o,i] and everything is kept transposed on-chip
([feature, batch] layout) so each layer's output feeds the next directly.

Sharding: data-parallel over the batch — 4096 tokens -> 512 per core; the
per-expert weight stacks are replicated. Matmuls run as float32r (full PE
rate, ~3e-4 rel err). PSUM accumulates over experts and the contraction; the
bias enters as one extra K=8 matmul (blend.T as rhs) per output tile.
Activations are stored 4 k-blocks wide ([128, 2048]) so one DVE prescale op
(with a free-dim-broadcast blend operand) feeds 4 k-tiles of matmuls.
Weights are pre-split by o-half on the host so every tile DMA is one
contiguous 512KB block.
"""

import os

import numpy as np
import ml_dtypes

import concourse.bass as bass
import concourse.tile as tile
import concourse.mybir as mybir
from concourse import bacc
from concourse.bass_utils import run_bass_kernel_spmd
from contextlib import ExitStack

dt = mybir.dt
ALU = mybir.AluOpType
ACTF = mybir.ActivationFunctionType

N_CORES = 8
B_FULL = 4096
BC = B_FULL // N_CORES  # 512 tokens per core
E = 8
DIMS = [1024, 2048, 2048, 512]
LAYERS = [  # (I, O, has_elu)
    (1024, 2048, True),
    (2048, 2048, True),
    (2048, 512, False),
]
OH = 1024   # o-columns per half-pass (8 psum banks)
GW = 4      # k-blocks packed per wide activation tile
WIDE = GW * BC

USE_BF16 = os.environ.get("MOE_MM_DT", "f32r") == "bf16"
MM_DT = dt.bfloat16 if USE_BF16 else dt.float32r
MM_NP = ml_dtypes.bfloat16 if USE_BF16 else np.float32

_cache = {}


def _build(with_bias=True):
    nc = bacc.Bacc("TRN2", target_bir_lowering=False, debug=False,
                   num_devices=N_CORES)
    xTw = nc.declare_dram_parameter("xTw", [DIMS[0] // (128 * GW), 128, WIDE],
                                    dt.float32, isOutput=False)
    blT = nc.declare_dram_parameter("blT", [E, BC], MM_DT, isOutput=False)
    brep = nc.declare_dram_parameter("brep", [E, 128, BC], dt.float32, isOutput=False)
    # weights pre-split by o-half on the host so every [128, width] tile DMA
    # reads one fully contiguous block
    wf = [nc.declare_dram_parameter(f"w{l}f",
                                    [max(O // OH, 1), E * I, min(OH, O)],
                                    MM_DT, isOutput=False)
          for l, (I, O, _) in enumerate(LAYERS)]
    bf = [nc.declare_dram_parameter(f"b{l}f", [E, O], MM_DT, isOutput=False)
          for l, (I, O, _) in enumerate(LAYERS)]
    yT = nc.declare_dram_parameter("yT", [DIMS[3], BC], dt.float32, isOutput=True)

    tc = tile.TileContext(nc)
    with tc:
        with ExitStack() as ctx:
            const = ctx.enter_context(tc.tile_pool(name="const", bufs=1))
            act = ctx.enter_context(tc.tile_pool(name="act", bufs=1))
            xpool = ctx.enter_context(tc.tile_pool(name="xpool", bufs=3))
            wpool = ctx.enter_context(tc.tile_pool(name="wpool", bufs=10))
            tpool = ctx.enter_context(tc.tile_pool(name="tpool", bufs=2))
            ypool = ctx.enter_context(tc.tile_pool(name="ypool", bufs=2))
            pp = ctx.enter_context(tc.tile_pool(name="pp", bufs=8, space="PSUM"))

            if with_bias:
                blT_sb = const.tile([E, BC], MM_DT, tag="blT")
                nc.sync.dma_start(blT_sb[:], blT[:])
            bf_sb = [None, None, None]
            brep_sb = [None] * E

            # warm the PE clock gate (HAM) with throwaway matmuls on a
            # memset constant (no DMA dependency) while the first input and
            # weight DMAs are in flight, so the real stream starts at
            # 2.4 GHz; the psum tile recycles into the first half's bank set
            wsrc_f = const.tile([E, BC], dt.float32, tag="wsrc_f")
            nc.vector.memset(wsrc_f[:], 1.0)
            wsrc = const.tile([E, BC], MM_DT, tag="wsrc")
            nc.vector.tensor_copy(wsrc[:], wsrc_f[:])
            warm = pp.tile([128, BC], dt.float32, tag="ps")
            for _ in range(13):
                nc.tensor.matmul(warm[:], wsrc[:, :128], wsrc[:],
                                 start=True, stop=True)

            # layer 0 input, packed 4 k-blocks wide; tile g loaded lazily at
            # first use so startup only waits for tile 0 (which is itself
            # split into chunks so the first prescale can begin early)
            hT = [None] * (DIMS[0] // (128 * GW))

            def load_x(g):
                t = act.tile([128, WIDE], dt.float32, name=f"h0_{g}", tag=f"h0_{g}")
                if g == 0:
                    for c4 in range(GW):
                        nc.sync.dma_start(t[:, c4 * BC:(c4 + 1) * BC],
                                          xTw[g, :, c4 * BC:(c4 + 1) * BC])
                else:
                    nc.sync.dma_start(t[:], xTw[g, :, :])
                hT[g] = t

            pending_drain = []  # deferred drain emission from the previous half

            for l, (I, O, has_elu) in enumerate(LAYERS):
                NG = I // (128 * GW)  # wide groups per layer input
                if with_bias:
                    t = const.tile([E, O], MM_DT, tag=f"bf{l}")
                    nc.sync.dma_start(t[:], bf[l][:])
                    bf_sb[l] = t
                h_next = []
                for g in range(O * BC // (128 * WIDE)):
                    h_next.append(act.tile([128, WIDE], dt.float32,
                                           name=f"h{l + 1}_{g}",
                                           tag=f"h{l + 1}_{g}"))
                for half_start in range(0, O, OH):
                    width = min(OH, O - half_start)
                    n_ot = width // 128
                    # open accumulation groups with the bias matmul (K=8);
                    # without bias the first weight matmul opens the group
                    ps = []
                    for j in range(n_ot):
                        p = pp.tile([128, BC], dt.float32, tag="ps")
                        if with_bias:
                            nc.tensor.matmul(
                                p[:],
                                bf_sb[l][:, half_start + j * 128:
                                         half_start + (j + 1) * 128],
                                blT_sb[:],
                                start=True, stop=False)
                        ps.append(p)
                    # stream experts x wide k-groups
                    for e in range(E):
                        if brep_sb[e] is None:
                            t = const.tile([128, BC], dt.float32, tag=f"brep{e}")
                            nc.sync.dma_start(t[:], brep[e, :, :])
                            brep_sb[e] = t
                        for g in range(NG):
                            if l == 0 and hT[g] is None:
                                load_x(g)
                            xp = xpool.tile([128, WIDE], MM_DT, tag="xp")
                            if l == 0 and e == 0 and g == 0:
                                # sliced prescale: each chunk only waits for
                                # its own quarter of the xT DMA
                                for c4 in range(GW):
                                    nc.vector.tensor_tensor(
                                        xp[:, c4 * BC:(c4 + 1) * BC],
                                        hT[g][:, c4 * BC:(c4 + 1) * BC],
                                        brep_sb[e][:], ALU.mult)
                            else:
                                nc.vector.tensor_tensor(
                                    xp[:].rearrange("p (c b) -> p c b", c=GW),
                                    hT[g][:].rearrange("p (c b) -> p c b", c=GW),
                                    brep_sb[e][:].unsqueeze(1).broadcast_to(
                                        (128, GW, BC)),
                                    ALU.mult)
                            last_g = (e == E - 1 and g == NG - 1)
                            wts = []
                            for c in range(GW):
                                kt = g * GW + c
                                wt = wpool.tile([128, width], MM_DT, tag="wt")
                                row = e * I + kt * 128
                                nc.sync.dma_start(
                                    wt[:], wf[l][half_start // OH,
                                                 row:row + 128, :])
                                wts.append(wt)
                                opener = (not with_bias and e == 0 and g == 0
                                          and c == 0)
                                if not last_g:
                                    for j in range(n_ot):
                                        nc.tensor.matmul(
                                            ps[j][:], wt[:, j * 128:(j + 1) * 128],
                                            xp[:, c * BC:(c + 1) * BC],
                                            start=opener, stop=False)
                            if last_g:
                                # final group: j-outer so each bank's group
                                # stops early and its drain overlaps the rest
                                for j in range(n_ot):
                                    for c in range(GW):
                                        nc.tensor.matmul(
                                            ps[j][:], wts[c][:, j * 128:(j + 1) * 128],
                                            xp[:, c * BC:(c + 1) * BC],
                                            start=False, stop=(c == GW - 1))
                            if e == 0 and g == 0 and pending_drain:
                                # emit the previous half's drains only after
                                # this half's first wave, so the scheduler
                                # prioritizes restarting the PE pipeline
                                for fn in pending_drain:
                                    fn()
                                pending_drain = []
                    # defer drain emission
                    def make_drain(l, has_elu, half_start, n_ot, ps, h_next):
                        def drain():
                            for j in range(n_ot):
                                ot = (half_start + j * 128) // 128
                                if has_elu:
                                    # elu(v) = relu(v) + exp(min(v,0)) - 1
                                    m = tpool.tile([128, BC], dt.float32, tag="m")
                                    nc.vector.tensor_scalar_min(m[:], ps[j][:], 0.0)
                                    r = tpool.tile([128, BC], dt.float32, tag="r")
                                    nc.scalar.activation(r[:], ps[j][:], ACTF.Relu)
                                    x2 = tpool.tile([128, BC], dt.float32, tag="x2")
                                    nc.scalar.activation(x2[:], m[:], ACTF.Exp)
                                    dst = h_next[ot // GW][
                                        :, (ot % GW) * BC:(ot % GW + 1) * BC]
                                    nc.vector.scalar_tensor_tensor(
                                        dst, x2[:], -1.0, r[:], ALU.add, ALU.add)
                                else:
                                    y = ypool.tile([128, BC], dt.float32, tag="y")
                                    nc.vector.tensor_copy(y[:], ps[j][:])
                                    nc.sync.dma_start(
                                        yT[half_start + j * 128:
                                           half_start + (j + 1) * 128, :],
                                        y[:])
                        return drain
                    pending_drain.append(
                        make_drain(l, has_elu, half_start, n_ot, ps, h_next))
                hT = h_next
            for fn in pending_drain:
                fn()
    nc.compile()
    return nc


def _prep_inputs(weight_blend, x, W0, B0, W1, B1, W2, B2):
    Ws = [W0, W1, W2]
    Bs = [B0, B1, B2]
    shared = {}
    for l in range(3):
        I, O, _ = LAYERS[l]
        wfl = Ws[l].transpose(0, 2, 1).reshape(E * I, O)
        nh = max(O // OH, 1)
        shared[f"w{l}f"] = np.ascontiguousarray(
            np.stack([wfl[:, h * OH:h * OH + min(OH, O)] for h in range(nh)]),
            dtype=MM_NP)
        shared[f"b{l}f"] = np.ascontiguousarray(Bs[l][:, :, 0], dtype=MM_NP)
    in_maps = []
    for c in range(N_CORES):
        s = slice(c * BC, (c + 1) * BC)
        blT = np.ascontiguousarray(weight_blend[s].T, dtype=np.float32)
        m = dict(shared)
        # pack x.T into [NG, 128, GW*BC] wide tiles: block kt = g*GW + c
        xt = np.ascontiguousarray(x[s].T, dtype=np.float32)  # [1024, 512]
        m["xTw"] = np.ascontiguousarray(
            xt.reshape(-1, GW, 128, BC).transpose(0, 2, 1, 3).reshape(-1, 128, WIDE))
        m["blT"] = blT.astype(MM_NP)
        m["brep"] = np.ascontiguousarray(
            np.broadcast_to(blT[:, None, :], (E, 128, BC)), dtype=np.float32)
        in_maps.append(m)
    return in_maps


def run(inputs, trace=False, tmpdir=None, trace_cores=None):
    """Run on hardware; returns (y, BassKernelResults)."""
    with_bias = any(
        np.any(np.asarray(inputs[k])) for k in ("B0", "B1", "B2"))
    key = ("nc", with_bias)
    if key not in _cache:
        _cache[key] = _build(with_bias)
    nc = _cache[key]
    in_maps = _prep_inputs(**inputs)
    kw = {}
    if tmpdir:
        kw["tmpdir"] = tmpdir
    if trace_cores:
        kw["trace_cores"] = trace_cores
    res = run_bass_kernel_spmd(
        nc, in_maps, core_ids=list(range(N_CORES)), trace=trace, **kw)
    y = np.concatenate([r["yT"].T for r in res.results], axis=0)
    return np.ascontiguousarray(y, dtype=np.float32), res


def kernel(**inputs):
    y, _ = run(inputs, trace=False)
    return y
